# revision 15
# baseline (speedup 1.0000x reference)
"""DCNv2 (deformable conv v2) Trainium2 kernel.

Data-parallel over batch: 1 image per NeuronCore, 8 cores.

Per-core pipeline:
  S1  load inputs (channel-major x, padded x for the offset conv, weights).
  S2  offset conv = 9 accumulating K=64 fp32r GEMMs over shifted views of
      padded x -> om [27, 9216].
  S3  PE-transpose om to pixel-major tiles (pixel i at partition i//72,
      block i%72 -- "p-major" so per-partition DRAM runs are contiguous).
  S4  pixel-major elementwise post: sigmoid(mask), sampling positions,
      floor/frac, clamps, validity, bilinear pair weights beta, and flat
      gather indices idx = 1 + clip(y)*96 + clip(x, -1, 95).
  S4b wrap the 18 index maps (tap x corner-row) into the SWDGE dma_gather
      index format ([16, 576] int16 wrapped, replicated to 128 partitions):
      PE-transpose -> DRAM write in wrap order (32B runs) -> cast readback.
  S5  per-tap 1x1 GEMMs U_k[1 + flat, oc] built pixel-major on PE
      (stationary x chunk, all 9 taps batched as N=512 + N=256 fp32r
      matmuls), ACT-evacuated, DMA'd to DRAM [9218, 64] with guard rows.
  S6  per (tap, corner-row, half): dma_gather with elem_size=128,
      elem_step=64 fetches overlapping 512B horizontal corner pairs.
  S7  DVE/GPSIMD bilinear combine, per-pixel weights broadcast along the
      channel dim via stride-0 APs; accumulate over taps into out^T.
  S8  out^T [9216, 64] to DRAM; host transposes back to [64, 96, 96].
"""

import contextlib
import sys

sys.path.insert(0, "/opt/trn_rl_repo")

import numpy as np

import concourse.bass as bass
import concourse.tile as tile
from concourse import bacc, mybir
from concourse.bass import AP

F32 = mybir.dt.float32
F32R = mybir.dt.float32r
I16 = mybir.dt.int16
I32 = mybir.dt.int32
ALU = mybir.AluOpType
ACTF = mybir.ActivationFunctionType

H = W = 96
HP = WP = 98
NPIX = H * W            # 9216
P = 128
NB = NPIX // P          # 72
CIN = COUT = 64
KK = 9
KK2 = 12                # taps padded to 12 for N>=256 fp32r matmuls
NOFF = 27
UROWS = NPIX + 2        # guard row each side
NL = 2 * KK             # 18 gather lists (tap x corner-row)
NW = NPIX // 16         # 576 wrapped columns


def _view(t, off_elems, dims):
    base = t[:]
    return AP(base.tensor, base.offset + off_elems, [base.ap[0]] + dims)


def build_nc(n_gpsimd_taps=2, om_rows_per_chunk=4, repeats=1):
    nc = bacc.Bacc(None, target_bir_lowering=False)

    x_pad = nc.dram_tensor("x_pad", [CIN, HP * WP], F32R, kind="ExternalInput")
    x_c = nc.dram_tensor("x_c", [CIN, NPIX], F32R, kind="ExternalInput")
    w_off = nc.dram_tensor("w_off", [CIN, KK, NOFF], F32R, kind="ExternalInput")
    w_dcn = nc.dram_tensor("w_dcn", [CIN, KK2, COUT], F32R, kind="ExternalInput")
    hmap = nc.dram_tensor("hmap", [P, NB], F32, kind="ExternalInput")
    wmap = nc.dram_tensor("wmap", [P, NB], F32, kind="ExternalInput")
    kyc = nc.dram_tensor("kyc", [P, KK], F32, kind="ExternalInput")
    kxc = nc.dram_tensor("kxc", [P, KK], F32, kind="ExternalInput")
    ident = nc.dram_tensor("ident", [P, P], F32, kind="ExternalInput")
    out_t = nc.dram_tensor("out_t", [NPIX, COUT], F32, kind="ExternalOutput")
    u_drams = [
        nc.dram_tensor(f"u{k}", [UROWS, COUT], F32, kind="Internal")
        for k in range(KK)
    ]
    idx_dram = nc.dram_tensor("idxw", [NL, 16, NW], F32, kind="Internal")

    with tile.TileContext(nc) as tc:
      def _emit(sfx):
       with contextlib.ExitStack() as ctx:
        consts = ctx.enter_context(tc.tile_pool(name="consts" + sfx, bufs=1))
        live = ctx.enter_context(tc.tile_pool(name="live" + sfx, bufs=1))

        beta0 = live.tile([P, NB, KK, 2], F32)
        beta1 = live.tile([P, NB, KK, 2], F32)
        idxw = live.tile([P, NL, NW], I16)  # wrapped, replicated x8
        vacc = live.tile([P, NB, COUT], F32)

        wofft = consts.tile([CIN, KK, NOFF], F32R)
        nc.sync.dma_start(out=wofft[:], in_=w_off[:])
        wdcnt = consts.tile([CIN, KK2, COUT], F32R)
        nc.sync.dma_start(out=wdcnt[:], in_=w_dcn[:])
        hm = consts.tile([P, NB], F32)
        nc.sync.dma_start(out=hm[:], in_=hmap[:])
        wm = consts.tile([P, NB], F32)
        nc.sync.dma_start(out=wm[:], in_=wmap[:])
        kyt = consts.tile([P, KK], F32)
        nc.sync.dma_start(out=kyt[:], in_=kyc[:])
        kxt = consts.tile([P, KK], F32)
        nc.sync.dma_start(out=kxt[:], in_=kxc[:])
        idt = consts.tile([P, P], F32)
        nc.sync.dma_start(out=idt[:], in_=ident[:])

        # zero U guard rows (clamped pairs read them with zero weight)
        zrow = consts.tile([1, COUT], F32)
        nc.vector.memset(zrow[:], 0.0)
        for k in range(KK):
            nc.sync.dma_start(out=u_drams[k][0:1, :], in_=zrow[:])
            nc.sync.dma_start(out=u_drams[k][UROWS - 1 : UROWS, :], in_=zrow[:])

        with tc.tile_pool(name="om_p" + sfx, bufs=1) as om_p:
            om = om_p.tile([NOFF, NPIX], F32)

            # ---------------- S2: offset conv ----------------
            with tc.tile_pool(name="xp_p" + sfx, bufs=1) as xp_p:
                xp = xp_p.tile([CIN, HP, WP], F32R)
                nc.sync.dma_start(
                    out=xp[:], in_=x_pad[:].rearrange("c (h w) -> c h w", h=HP)
                )
                RPC = om_rows_per_chunk
                with tc.tile_pool(name="ompsum" + sfx, bufs=4, space="PSUM") as omp:
                    for chi in range(H // RPC):
                        y0 = chi * RPC
                        ps = omp.tile([NOFF, RPC * W], F32)
                        for k in range(KK):
                            ky, kx = k // 3, k % 3
                            rhs = xp[:, y0 + ky : y0 + ky + RPC, kx : kx + W]
                            nc.tensor.matmul(
                                ps[:],
                                wofft[:, k, :],
                                rhs,
                                start=(k == 0),
                                stop=(k == KK - 1),
                            )
                        nc.scalar.copy(
                            out=om[:, y0 * W : (y0 + RPC) * W], in_=ps[:]
                        )

            # ---------------- S3 + S4: transpose + post ----------------
            with tc.tile_pool(name="post" + sfx, bufs=1) as post_p:
                omt = post_p.tile([P, NB, NOFF], F32)
                TPB = 18
                with tc.tile_pool(name="tppsum" + sfx, bufs=2, space="PSUM") as tpp:
                    for g_ in range(NB // TPB):
                        ps = tpp.tile([P, TPB, NOFF], F32)
                        for t in range(TPB):
                            b = g_ * TPB + t
                            src = _view(om, b, [[NB, P]])  # cols {p*NB+b}
                            nc.tensor.transpose(
                                ps[:, t, :], src, idt[0:NOFF, 0:NOFF]
                            )
                        nc.scalar.copy(
                            out=omt[:, g_ * TPB : (g_ + 1) * TPB, :], in_=ps[:]
                        )

                dy = _view(omt, 0, [[NOFF, NB], [2, KK]])
                dx = _view(omt, 1, [[NOFF, NB], [2, KK]])
                mlog = _view(omt, 18, [[NOFF, NB], [1, KK]])

                _tagn = [0]

                def t3():
                    _tagn[0] += 1
                    return post_p.tile(
                        [P, NB, KK], F32, tag=f"t3_{_tagn[0]}{sfx}",
                        name=f"t3_{_tagn[0]}{sfx}",
                    )

                def bc_tap(t):
                    return _view(t, 0, [[t[:].ap[1][0], NB], [0, KK]])

                def bc_blk(t):
                    return _view(t, 0, [[0, NB], [t[:].ap[1][0], KK]])

                msk = t3()
                nc.scalar.activation(out=msk[:], in_=mlog, func=ACTF.Sigmoid)

                py = t3()
                nc.vector.tensor_add(py[:], dy, bc_tap(hm))
                nc.vector.tensor_add(py[:], py[:], bc_blk(kyt))
                px = t3()
                nc.vector.tensor_add(px[:], dx, bc_tap(wm))
                nc.vector.tensor_add(px[:], px[:], bc_blk(kxt))

                def floor_(src):
                    ti = post_p.tile(
                        [P, NB, KK], I32, tag="flr_i" + sfx, name="flr_i" + sfx, bufs=2
                    )
                    nc.vector.tensor_copy(out=ti[:], in_=src[:])
                    tf = t3()
                    nc.vector.tensor_copy(out=tf[:], in_=ti[:])
                    fx = post_p.tile(
                        [P, NB, KK], F32, tag="flr_f" + sfx, name="flr_f" + sfx, bufs=2
                    )
                    nc.vector.tensor_tensor(fx[:], tf[:], src[:], op=ALU.is_gt)
                    nc.vector.tensor_sub(tf[:], tf[:], fx[:])
                    return tf

                yf = floor_(py)
                xf = floor_(px)
                ly = t3()
                nc.vector.tensor_sub(ly[:], py[:], yf[:])
                lx = t3()
                nc.vector.tensor_sub(lx[:], px[:], xf[:])

                def clamp(src, lo, hi):
                    o = t3()
                    nc.vector.tensor_scalar(
                        o[:], src[:], lo, hi, op0=ALU.max, op1=ALU.min
                    )
                    return o

                yc0 = clamp(yf, 0.0, 95.0)
                yc1m = clamp(yf, -1.0, 94.0)
                xcg = clamp(xf, -1.0, 95.0)
                xc0v = clamp(xf, 0.0, 95.0)
                xc1v = clamp(xf, -1.0, 94.0)

                def eqmask(a, bt):
                    o = t3()
                    nc.vector.tensor_tensor(o[:], a[:], bt[:], op=ALU.is_equal)
                    return o

                vy0 = eqmask(yc0, yf)
                vy1 = eqmask(yc1m, yf)
                vx0 = eqmask(xc0v, xf)
                vx1 = eqmask(xc1v, xf)

                # gather list values (f32): idx0 = yc0*96 + xcg + 1,
                # idx1 = (yc1m+1)*96 + xcg + 1
                tf0 = t3()
                nc.vector.tensor_scalar(
                    tf0[:], yc0[:], 96.0, 1.0, op0=ALU.mult, op1=ALU.add
                )
                nc.vector.tensor_add(tf0[:], tf0[:], xcg[:])
                tf1 = t3()
                nc.vector.tensor_scalar(
                    tf1[:], yc1m[:], 96.0, 97.0, op0=ALU.mult, op1=ALU.add
                )
                nc.vector.tensor_add(tf1[:], tf1[:], xcg[:])

                a0 = t3()
                nc.vector.tensor_scalar(
                    a0[:], ly[:], -1.0, 1.0, op0=ALU.mult, op1=ALU.add
                )
                nc.vector.tensor_mul(a0[:], a0[:], msk[:])
                nc.vector.tensor_mul(a0[:], a0[:], vy0[:])
                a1 = t3()
                nc.vector.tensor_mul(a1[:], ly[:], msk[:])
                nc.vector.tensor_mul(a1[:], a1[:], vy1[:])
                b0 = t3()
                nc.vector.tensor_scalar(
                    b0[:], lx[:], -1.0, 1.0, op0=ALU.mult, op1=ALU.add
                )
                nc.vector.tensor_mul(b0[:], b0[:], vx0[:])
                b1 = t3()
                nc.vector.tensor_mul(b1[:], lx[:], vx1[:])

                nc.vector.tensor_mul(beta0[:, :, :, 0], a0[:], b0[:])
                nc.vector.tensor_mul(beta0[:, :, :, 1], a0[:], b1[:])
                nc.vector.tensor_mul(beta1[:, :, :, 0], a1[:], b0[:])
                nc.vector.tensor_mul(beta1[:, :, :, 1], a1[:], b1[:])

                # ---------------- S4b: wrap the index lists ----------------
                # list L = 2k + rho; idx_dram[L][r, j] = LIST_L[16j + r] where
                # LIST_L[n] = idxval(pixel (n%128)*72 + n//128). Tst[bb, L, pp]
                # = idxval(pixel pp*72 + bb); j = 8*bb + q reads pp = 16q + r.
                tst = post_p.tile([NB, NL, P], F32)
                with tc.tile_pool(name="txpsum" + sfx, bufs=2, space="PSUM") as txp:
                    for L in range(NL):
                        k, rho = L // 2, L % 2
                        srcm = [tf0, tf1][rho]
                        ps = txp.tile([NB, P], F32, tag="tx" + sfx, name="tx" + sfx)
                        nc.tensor.transpose(
                            ps[:],
                            _view(srcm, k, [[KK, NB]]),  # [P, NB] k-slice
                            idt[:],
                        )
                        # evac permuted so tst[bb, L, r*8 + q] = ps[bb, 16q+r]
                        psv = AP(
                            ps.tensor, ps[:].offset,
                            [ps[:].ap[0], [16, 8], [1, 16]],
                        )
                        nc.scalar.copy(
                            out=_view(tst, L * P, [[1, 8], [8, 16]]), in_=psv
                        )
                # per-list DMA, 8-elem (32B) runs:
                # dram[L][r][8*bb + q] <- Tst[bb, L, 16q + r]
                for L in range(NL):
                    dst = AP(
                        idx_dram,
                        L * 16 * NW,
                        [[8, NB], [NW, 16], [1, 8]],
                    )
                    src_w = _view(tst, L * P, [[1, P]])
                    nc.sync.dma_start(out=dst, in_=src_w)
                # cast readback into wrapped int16, one DMA per group
                for g_ in range(8):
                    rep_in = AP(
                        idx_dram,
                        0,
                        [[NW, 16], [16 * NW, NL], [1, NW]],
                    )
                    nc.gpsimd.dma_start(
                        out=idxw[16 * g_ : 16 * (g_ + 1), :, :], in_=rep_in
                    )

        # ---------------- S5: U GEMMs -> DRAM ----------------
        NSC = 9
        NSLAB = NB // NSC
        with (
            tc.tile_pool(name="xc_p" + sfx, bufs=1) as xc_p,
            tc.tile_pool(name="ustage" + sfx, bufs=2) as ust_p,
            tc.tile_pool(name="upsum1" + sfx, bufs=3, space="PSUM") as ups1_p,
            tc.tile_pool(name="upsum2" + sfx, bufs=3, space="PSUM") as ups2_p,
        ):
            xc = xc_p.tile([CIN, NPIX], F32R)
            nc.sync.dma_start(out=xc[:], in_=x_c[:])
            for s in range(NSLAB):
                stg = ust_p.tile([P, NSC, KK, COUT], F32, tag="stg" + sfx, name="stg" + sfx)
                for cl in range(NSC):
                    b = s * NSC + cl
                    lhsT = _view(xc, b, [[NB, P]])  # cols {p*NB + b}
                    ps1 = ups1_p.tile([P, 8 * COUT], F32, tag="ps1" + sfx, name="ps1" + sfx)
                    ps2 = ups2_p.tile([P, 4 * COUT], F32, tag="ps2" + sfx, name="ps2" + sfx)
                    nc.tensor.matmul(
                        ps1[:], lhsT, wdcnt[:, 0:8, :], start=True, stop=True
                    )
                    nc.tensor.matmul(
                        ps2[:], lhsT, wdcnt[:, 8:12, :], start=True, stop=True
                    )
                    nc.scalar.copy(out=stg[:, cl, 0:8, :], in_=ps1[:])
                    nc.scalar.copy(out=stg[:, cl, 8, :], in_=ps2[:, 0:COUT])
                for k in range(KK):
                    # partition p writes rows {1 + p*NB + s*NSC + cl}
                    dst = AP(
                        u_drams[k],
                        (1 + s * NSC) * COUT,
                        [[NB * COUT, P], [COUT, NSC], [1, COUT]],
                    )
                    nc.sync.dma_start(
                        out=dst,
                        in_=_view(
                            stg, k * COUT, [[KK * COUT, NSC], [1, COUT]]
                        ),
                    )

        # ---------------- S6/S7: gathers + bilinear combine ----------------
        NHB = NB // 2
        NIH = NHB * P  # 4608 indices per half
        with tc.tile_pool(name="gpool" + sfx, bufs=2) as gp:
            for half in range(2):
                bs = half * NHB
                for k in range(KK):
                    g0 = gp.tile([P, NHB, 2 * COUT], F32, tag="g0" + sfx, name="g0" + sfx)
                    g1 = gp.tile([P, NHB, 2 * COUT], F32, tag="g1" + sfx, name="g1" + sfx)
                    for rho, gt in ((0, g0), (1, g1)):
                        L = 2 * k + rho
                        nc.gpsimd.dma_gather(
                            out_ap=gt[:],
                            in_ap=AP(
                                u_drams[k],
                                0,
                                [[COUT, UROWS - 1], [1, 2 * COUT]],
                            ),
                            idxs_ap=idxw[
                                :, L, half * (NW // 2) : (half + 1) * (NW // 2)
                            ],
                            num_idxs=NIH,
                            num_idxs_reg=NIH,
                            elem_size=2 * COUT,
                            elem_step=COUT,
                            single_packet=False,
                        )
                    gv0 = _view(g0, 0, [[2 * COUT, NHB], [1, COUT], [COUT, 2]])
                    gv1 = _view(g1, 0, [[2 * COUT, NHB], [1, COUT], [COUT, 2]])

                    def bv(bt):
                        base = bt[:]
                        return AP(
                            base.tensor,
                            base.offset + bs * (KK * 2) + k * 2,
                            [base.ap[0], [KK * 2, NHB], [0, COUT], [1, 2]],
                        )

                    eng = nc.gpsimd if k >= KK - n_gpsimd_taps else nc.vector
                    eng.tensor_mul(gv0, gv0, bv(beta0))
                    eng.tensor_mul(gv1, gv1, bv(beta1))
                    eng.tensor_add(gv0, gv0, gv1)
                    s0 = _view(g0, 0, [[2 * COUT, NHB], [1, COUT]])
                    s1 = _view(g0, COUT, [[2 * COUT, NHB], [1, COUT]])
                    vs = vacc[:, bs : bs + NHB, :]
                    if k == 0:
                        eng.tensor_add(vs, s0, s1)
                    else:
                        tmp = gp.tile(
                            [P, NHB, COUT], F32, tag="lerptmp" + sfx, name="lerptmp" + sfx
                        )
                        eng.tensor_add(tmp[:], s0, s1)
                        nc.vector.tensor_add(vs, vs, tmp[:])

        # ---------------- S8 ----------------
        nc.sync.dma_start(
            out=out_t[:].rearrange("(p b) o -> p b o", p=P), in_=vacc[:]
        )

      for r in range(repeats):
          _emit(f"_{r}" if repeats > 1 else "")

    nc.compile()
    return nc


def build_nc_repeat(repeats):
    return build_nc(repeats=repeats)


_NC_CACHE = {}


def _get_nc():
    if "nc" not in _NC_CACHE:
        _NC_CACHE["nc"] = build_nc()
    return _NC_CACHE["nc"]


def make_host_inputs(x, w_offset, w_dcn):
    x = np.asarray(x, np.float32)
    w_offset = np.asarray(w_offset, np.float32)
    w_dcn = np.asarray(w_dcn, np.float32)
    w_off_r = np.ascontiguousarray(
        w_offset.reshape(NOFF, CIN, KK).transpose(1, 2, 0)
    )
    w_dcn_r = np.zeros((CIN, KK2, COUT), np.float32)
    w_dcn_r[:, :KK, :] = w_dcn.reshape(COUT, CIN, KK).transpose(1, 2, 0)
    kyc = np.broadcast_to(
        (np.arange(KK) // 3 - 1).astype(np.float32), (P, KK)
    ).copy()
    kxc = np.broadcast_to(
        (np.arange(KK) % 3 - 1).astype(np.float32), (P, KK)
    ).copy()
    ii = np.arange(P)[:, None] * NB + np.arange(NB)[None, :]
    hmap = (ii // W).astype(np.float32)
    wmap = (ii % W).astype(np.float32)
    ident = np.eye(P, dtype=np.float32)

    in_maps = []
    for b in range(x.shape[0]):
        x_pad = np.zeros((CIN, HP, WP), np.float32)
        x_pad[:, 1:97, 1:97] = x[b]
        in_maps.append(
            {
                "x_pad": x_pad.reshape(CIN, HP * WP),
                "x_c": np.ascontiguousarray(x[b].reshape(CIN, NPIX)),
                "w_off": w_off_r,
                "w_dcn": w_dcn_r,
                "kyc": kyc,
                "kxc": kxc,
                "hmap": hmap,
                "wmap": wmap,
                "ident": ident,
            }
        )
    return in_maps


def assemble_output(results, B):
    return np.stack(
        [results[b]["out_t"].T.reshape(COUT, H, W) for b in range(B)]
    )


def kernel(x, w_offset, w_dcn):
    B = x.shape[0]
    assert B == 8
    in_maps = make_host_inputs(x, w_offset, w_dcn)
    from concourse.bass_utils import run_bass_kernel_spmd

    nc = _get_nc()
    res = run_bass_kernel_spmd(nc, in_maps, core_ids=list(range(B)))
    return assemble_output(res.results, B)
